# revision 18
# baseline (speedup 1.0000x reference)
"""LSTMCell (B=16384, I=H=512) on 8 Trainium2 NeuronCores.

Strategy: data-parallel over the batch (2048 rows/core). Each core computes
gatesT = W @ [x;h]T in transposed layout (gate dim on partitions, batch on the
free dim) so that:
  - the contraction dim (I+H) lands on SBUF partitions for both matmul
    operands with zero on-chip transposes (inputs are pre-transposed on the
    host while sharding),
  - the gate bias is a per-partition vector, applied for free by the ScalarE
    activation instruction.
All matmul operands are bf16 (host-cast; rel err ~1e-3 vs the 2e-2 budget),
which runs the PE at the same rate as fp32r but halves HBM traffic and SBUF
footprint. That headroom buys full double-buffering: weights and activations
for rep N+1 stream while rep N computes, in 512KB DMAs ([128, 2048] per
contraction slice, x and h fused into one xhT tensor).
The stacked gate dim is permuted on the host so each 128-row h-block's four
gate tiles (i, f, g, o) are contiguous in the weight matrix.
Elementwise LSTM tail (sigmoid/tanh/mul/add) runs on ScalarE + VectorE
overlapped with the matmuls; c loads and bf16 result stores ride the scalar
HWDGE ring so they never queue behind the big sync-ring loads. Outputs are
stored transposed in bf16 and un-transposed/upcast on the host.
"""

import numpy as np
from contextlib import ExitStack

_B, _I, _H = 16384, 512, 512
_NC = 8
_BL = _B // _NC          # 2048 batch rows per core
_G = 4 * _H              # 2048 stacked gate dim
_K = _I + _H             # 1024 contraction dim
_BCH = 512               # batch chunk (PSUM bank free size)
_NB = _BL // _BCH        # 4 batch chunks
_NJ = _H // 128          # 4 h-blocks of 128
_NK = _K // 128          # 8 k-chunks of 128
_NT = 4                  # gates (i, f, g, o)

_cache = {}


def _build(reps=1, unroll=False):
    from concourse import bacc
    import concourse.mybir as mybir
    import concourse.tile as tile

    f32 = mybir.dt.float32
    bf16 = mybir.dt.bfloat16
    AF = mybir.ActivationFunctionType

    nc = bacc.Bacc("TRN2", target_bir_lowering=False, debug=False,
                   num_devices=_NC)
    xhT = nc.declare_dram_parameter("xhT", [_K, _BL], bf16, isOutput=False)
    cT = nc.declare_dram_parameter("cT", [_H, _BL], bf16, isOutput=False)
    # gate dim pre-permuted on host: column block j*512..j*512+512 holds the
    # (i, f, g, o) tiles for h-block j, each 128 wide.
    wT = nc.declare_dram_parameter("wT", [_K, _G], bf16, isOutput=False)
    b2 = nc.declare_dram_parameter("b2", [128, _G // 128], f32, isOutput=False)
    hoT = nc.declare_dram_parameter("hoT", [_H, _BL], bf16, isOutput=True)
    coT = nc.declare_dram_parameter("coT", [_H, _BL], bf16, isOutput=True)

    with ExitStack() as ctx:
        tc = ctx.enter_context(tile.TileContext(nc))
        wp = ctx.enter_context(tc.tile_pool(name="w", bufs=2))
        xp = ctx.enter_context(tc.tile_pool(name="xh", bufs=2))
        bp = ctx.enter_context(tc.tile_pool(name="bias", bufs=2))
        cp = ctx.enter_context(tc.tile_pool(name="cin", bufs=2))
        ap = ctx.enter_context(tc.tile_pool(name="act", bufs=2))
        op = ctx.enter_context(tc.tile_pool(name="out", bufs=2))
        pp = ctx.enter_context(tc.tile_pool(name="ps", bufs=2, space="PSUM"))

        wT_v = wT.rearrange("(k p) g -> p k g", p=128)
        xhT_v = xhT.rearrange("(k p) b -> p k b", p=128)

        def alloc_set():
            s = {"c": [None] * _NJ}
            s["w"] = wp.tile([128, _NK, _G], bf16, tag="w", name="w")
            s["xh"] = xp.tile([128, _NK, _BL], bf16, tag="xh", name="xh")
            for j in range(_NJ):
                s["c"][j] = cp.tile([128, _BL], bf16, tag=f"c{j}",
                                    name=f"c{j}")
            s["bias"] = bp.tile([128, _G // 128], f32, tag="bias",
                                name="bias")
            return s

        def load_set(s):
            # two giant input DMAs on the sync ring; c + bias on scalar
            nc.sync.dma_start(out=s["w"][:], in_=wT_v[:])
            nc.sync.dma_start(out=s["xh"][:], in_=xhT_v[:])
            for j in range(_NJ):
                nc.scalar.dma_start(out=s["c"][j][:],
                                    in_=cT[j * 128:(j + 1) * 128, :])
            nc.scalar.dma_start(out=s["bias"][:], in_=b2[:])

        def compute(s):
            # (j, t) outer / bc inner: each 128x128 weight tile is stationary
            # for 4 consecutive matmuls (the 4 batch chunks), quartering the
            # LDWEIGHTS traffic on the PE. PSUM: one bank per bc, x2 buffers;
            # the ScalarE activations drain each bank while the PE streams
            # the next group, keeping the PE at its 2.4GHz issue rate.
            w_sb, xh_sb, c_sb, bias_sb = s["w"], s["xh"], s["c"], s["bias"]
            AFS = [AF.Sigmoid, AF.Sigmoid, AF.Tanh, AF.Sigmoid]
            for j in range(_NJ):
                gt = [[None] * _NB for _ in range(_NT)]
                for t in range(_NT):
                    ps = []
                    for bc in range(_NB):
                        pst = pp.tile([128, _BCH], f32, tag=f"ps{bc}")
                        ps.append(pst)
                    wcol = j * 512 + t * 128
                    for k in range(_NK):
                        for bc in range(_NB):
                            nc.tensor.matmul(
                                ps[bc][:],
                                w_sb[:, k, wcol:wcol + 128],
                                xh_sb[:, k, bc * _BCH:(bc + 1) * _BCH],
                                start=(k == 0), stop=(k == _NK - 1),
                            )
                    bias_ap = bias_sb[:, j * _NT + t:j * _NT + t + 1]
                    for bc in range(_NB):
                        g_ = ap.tile([128, _BCH], bf16, tag=f"g{t}_{bc}")
                        nc.scalar.activation(g_[:], ps[bc][:], AFS[t],
                                             bias=bias_ap)
                        gt[t][bc] = g_
                for bc in range(_NB):
                    bsl = slice(bc * _BCH, (bc + 1) * _BCH)
                    gI, gF, gG, gO = (gt[0][bc], gt[1][bc],
                                      gt[2][bc], gt[3][bc])
                    fc = op.tile([128, _BCH], bf16, tag="fc")
                    ig = op.tile([128, _BCH], bf16, tag="ig")
                    newc = op.tile([128, _BCH], bf16, tag="newc")
                    newh = op.tile([128, _BCH], bf16, tag="newh")
                    nc.vector.tensor_mul(fc[:], gF[:], c_sb[j][:, bsl])
                    nc.vector.tensor_mul(ig[:], gI[:], gG[:])     # i * g
                    nc.vector.tensor_add(newc[:], fc[:], ig[:])
                    nc.scalar.activation(gG[:], newc[:], AF.Tanh)
                    nc.vector.tensor_mul(newh[:], gO[:], gG[:])
                    nc.scalar.dma_start(out=coT[j * 128:(j + 1) * 128, bsl],
                                        in_=newc[:])
                    nc.scalar.dma_start(out=hoT[j * 128:(j + 1) * 128, bsl],
                                        in_=newh[:])

        if reps == 1:
            sA = alloc_set()
            load_set(sA)
            compute(sA)
        else:
            # Software pipeline: two resident input sets; each loop iteration
            # runs two reps, loading one set's next inputs while computing
            # from the other. The For_i back-edge all-engine barrier
            # (~2-4us) is amortized over two reps and never sits between a
            # load and its consumer.
            assert reps % 4 == 0, "pipelined timing build needs reps % 4 == 0"
            sA = alloc_set()
            sB = alloc_set()
            load_set(sA)

            def body(_iv=None):
                for _ in range(2):
                    load_set(sB)
                    compute(sA)
                    load_set(sA)
                    compute(sB)

            if unroll:
                for _ in range(reps // 4):
                    body()
            else:
                engines = tuple(mybir.ALL_ENGINES)
                with tc.For_i(0, reps // 4, 1, hint_engines=engines):
                    body()
    nc.compile()
    return nc


# Gate-dim permutation: position j*4 + t  <-  original gate tile t*4 + j
# (tile index into the stacked-gates dim of 16 x 128 rows).
def _gate_perm():
    perm = np.empty(_G, np.int64)
    pos = 0
    for j in range(_NJ):
        for t in range(_NT):
            src = (t * _NJ + j) * 128
            perm[pos:pos + 128] = np.arange(src, src + 128)
            pos += 128
    return perm


def _bf16():
    import ml_dtypes
    return ml_dtypes.bfloat16


def _host_shards(x, h, c, Wi, bi, Wh, bh):
    bf16 = _bf16()
    perm = _gate_perm()
    W = np.concatenate([np.asarray(Wi, np.float32),
                        np.asarray(Wh, np.float32)], axis=1)    # [G, K]
    wTv = np.ascontiguousarray(W[perm].T.astype(bf16))          # [K, G]
    b = (np.asarray(bi, np.float32) + np.asarray(bh, np.float32))[perm]
    b2 = np.ascontiguousarray(b.reshape(_G // 128, 128).T)      # [128, G/128]
    xh = np.concatenate([np.asarray(x, np.float32),
                         np.asarray(h, np.float32)], axis=1)    # [B, K]
    in_maps = []
    for s in range(_NC):
        sl = slice(s * _BL, (s + 1) * _BL)
        in_maps.append({
            "xhT": np.ascontiguousarray(xh[sl].T.astype(bf16)),
            "cT": np.ascontiguousarray(
                np.asarray(c, np.float32)[sl].T.astype(bf16)),
            "wT": wTv,
            "b2": b2,
        })
    return in_maps


def kernel(x, h, c, Wi, bi, Wh, bh):
    from concourse.bass_utils import run_bass_kernel_spmd

    nc = _cache.get("nc")
    if nc is None:
        nc = _build()
        _cache["nc"] = nc

    in_maps = _host_shards(x, h, c, Wi, bi, Wh, bh)
    res = run_bass_kernel_spmd(nc, in_maps, list(range(_NC)))

    h_out = np.empty((_B, _H), np.float32)
    c_out = np.empty((_B, _H), np.float32)
    for s in range(_NC):
        sl = slice(s * _BL, (s + 1) * _BL)
        h_out[sl] = res.results[s]["hoT"].astype(np.float32).T
        c_out[sl] = res.results[s]["coT"].astype(np.float32).T
    return h_out, c_out


# revision 21
# speedup vs baseline: 1.4948x; 1.4948x over previous
"""LSTMCell (B=16384, I=H=512) on 8 Trainium2 NeuronCores.

Strategy: data-parallel over the batch (2048 rows/core). Each core computes
gatesT = W @ [x;h]T in transposed layout (gate dim on partitions, batch on the
free dim) so that:
  - the contraction dim (I+H) lands on SBUF partitions for both matmul
    operands with zero on-chip transposes (inputs are pre-transposed on the
    host while sharding),
  - the gate bias is a per-partition vector, applied for free by the ScalarE
    activation instruction.
All matmul operands are bf16 (host-cast; rel err ~1e-3 vs the 2e-2 budget),
which runs the PE at the same rate as fp32r but halves HBM traffic and SBUF
footprint. That headroom buys full double-buffering: weights and activations
for rep N+1 stream while rep N computes, in 512KB DMAs ([128, 2048] per
contraction slice, x and h fused into one xhT tensor).
The stacked gate dim is permuted on the host so each 128-row h-block's four
gate tiles (i, f, g, o) are contiguous in the weight matrix.
Elementwise LSTM tail (sigmoid/tanh/mul/add) runs on ScalarE + VectorE
overlapped with the matmuls; c loads and bf16 result stores ride the scalar
HWDGE ring so they never queue behind the big sync-ring loads. Outputs are
stored transposed in bf16 and un-transposed/upcast on the host.
"""

import numpy as np
from contextlib import ExitStack

_B, _I, _H = 16384, 512, 512
_NC = 8
_BL = _B // _NC          # 2048 batch rows per core
_G = 4 * _H              # 2048 stacked gate dim
_K = _I + _H             # 1024 contraction dim
_BCH = 512               # batch chunk (PSUM bank free size)
_NB = _BL // _BCH        # 4 batch chunks
_NJ = _H // 128          # 4 h-blocks of 128
_NK = _K // 128          # 8 k-chunks of 128
_NT = 4                  # gates (i, f, g, o)

_cache = {}


def _build(reps=1, unroll=False):
    from concourse import bacc
    import concourse.mybir as mybir
    import concourse.tile as tile

    f32 = mybir.dt.float32
    bf16 = mybir.dt.bfloat16
    AF = mybir.ActivationFunctionType

    nc = bacc.Bacc("TRN2", target_bir_lowering=False, debug=False,
                   num_devices=_NC)
    xhT = nc.declare_dram_parameter("xhT", [_K, _BL], bf16, isOutput=False)
    cT = nc.declare_dram_parameter("cT", [_H, _BL], bf16, isOutput=False)
    # gate dim pre-permuted on host: column block j*512..j*512+512 holds the
    # (i, f, g, o) tiles for h-block j, each 128 wide.
    wT = nc.declare_dram_parameter("wT", [_K, _G], bf16, isOutput=False)
    b2 = nc.declare_dram_parameter("b2", [128, _G // 128], f32, isOutput=False)
    hoT = nc.declare_dram_parameter("hoT", [_H, _BL], bf16, isOutput=True)
    coT = nc.declare_dram_parameter("coT", [_H, _BL], bf16, isOutput=True)

    with ExitStack() as ctx:
        tc = ctx.enter_context(tile.TileContext(nc))
        wp = ctx.enter_context(tc.tile_pool(name="w", bufs=2))
        xp = ctx.enter_context(tc.tile_pool(name="xh", bufs=2))
        bp = ctx.enter_context(tc.tile_pool(name="bias", bufs=2))
        cp = ctx.enter_context(tc.tile_pool(name="cin", bufs=2))
        ap = ctx.enter_context(tc.tile_pool(name="act", bufs=2))
        op = ctx.enter_context(tc.tile_pool(name="out", bufs=2))
        pp = ctx.enter_context(tc.tile_pool(name="ps", bufs=2, space="PSUM"))

        wT_v = wT.rearrange("(k p) g -> p k g", p=128)
        xhT_v = xhT.rearrange("(k p) b -> p k b", p=128)

        def alloc_set():
            s = {"c": [None] * _NJ}
            s["w"] = wp.tile([128, _NK, _G], bf16, tag="w", name="w")
            s["xh"] = xp.tile([128, _NK, _BL], bf16, tag="xh", name="xh")
            for j in range(_NJ):
                s["c"][j] = cp.tile([128, _BL], bf16, tag=f"c{j}",
                                    name=f"c{j}")
            s["bias"] = bp.tile([128, _G // 128], f32, tag="bias",
                                name="bias")
            return s

        def load_set(s):
            # two giant input DMAs on the sync ring; c + bias on scalar
            nc.sync.dma_start(out=s["w"][:], in_=wT_v[:])
            nc.sync.dma_start(out=s["xh"][:], in_=xhT_v[:])
            for j in range(_NJ):
                nc.scalar.dma_start(out=s["c"][j][:],
                                    in_=cT[j * 128:(j + 1) * 128, :])
            nc.scalar.dma_start(out=s["bias"][:], in_=b2[:])

        def compute(s):
            # (j, t) outer / bc inner: each 128x128 weight tile is stationary
            # for 4 consecutive matmuls (the 4 batch chunks), quartering the
            # LDWEIGHTS traffic on the PE. PSUM: one bank per bc, x2 buffers;
            # the ScalarE activations drain each bank while the PE streams
            # the next group, keeping the PE at its 2.4GHz issue rate.
            w_sb, xh_sb, c_sb, bias_sb = s["w"], s["xh"], s["c"], s["bias"]
            AFS = [AF.Sigmoid, AF.Sigmoid, AF.Tanh, AF.Sigmoid]
            for j in range(_NJ):
                gt = [[None] * _NB for _ in range(_NT)]
                for t in range(_NT):
                    ps = []
                    for bc in range(_NB):
                        pst = pp.tile([128, _BCH], f32, tag=f"ps{bc}")
                        ps.append(pst)
                    wcol = j * 512 + t * 128
                    for k in range(_NK):
                        for bc in range(_NB):
                            nc.tensor.matmul(
                                ps[bc][:],
                                w_sb[:, k, wcol:wcol + 128],
                                xh_sb[:, k, bc * _BCH:(bc + 1) * _BCH],
                                start=(k == 0), stop=(k == _NK - 1),
                            )
                    bias_ap = bias_sb[:, j * _NT + t:j * _NT + t + 1]
                    for bc in range(_NB):
                        g_ = ap.tile([128, _BCH], bf16, tag=f"g{t}_{bc}")
                        nc.scalar.activation(g_[:], ps[bc][:], AFS[t],
                                             bias=bias_ap)
                        gt[t][bc] = g_
                for half in range(2):
                    cst = op.tile([128, 2 * _BCH], bf16, tag="cst",
                                  name="cst")
                    hst = op.tile([128, 2 * _BCH], bf16, tag="hst",
                                  name="hst")
                    for bc2 in range(2):
                        bc = half * 2 + bc2
                        bsl = slice(bc * _BCH, (bc + 1) * _BCH)
                        lsl = slice(bc2 * _BCH, (bc2 + 1) * _BCH)
                        gI, gF, gG, gO = (gt[0][bc], gt[1][bc],
                                          gt[2][bc], gt[3][bc])
                        nc.vector.tensor_mul(cst[:, lsl], gF[:],
                                             c_sb[j][:, bsl])
                        nc.vector.tensor_mul(hst[:, lsl], gI[:], gG[:])
                        nc.vector.tensor_add(cst[:, lsl], cst[:, lsl],
                                             hst[:, lsl])
                        nc.scalar.activation(gG[:], cst[:, lsl], AF.Tanh)
                        nc.vector.tensor_mul(hst[:, lsl], gO[:], gG[:])
                    osl = slice(half * 2 * _BCH, (half + 1) * 2 * _BCH)
                    nc.scalar.dma_start(out=coT[j * 128:(j + 1) * 128, osl],
                                        in_=cst[:])
                    nc.scalar.dma_start(out=hoT[j * 128:(j + 1) * 128, osl],
                                        in_=hst[:])

        if reps == 1:
            sA = alloc_set()
            load_set(sA)
            compute(sA)
        else:
            # Software pipeline: two resident input sets; each loop iteration
            # runs two reps, loading one set's next inputs while computing
            # from the other. The For_i back-edge all-engine barrier
            # (~2-4us) is amortized over two reps and never sits between a
            # load and its consumer.
            assert reps % 4 == 0, "pipelined timing build needs reps % 4 == 0"
            sA = alloc_set()
            sB = alloc_set()
            load_set(sA)

            def body(_iv=None):
                for _ in range(2):
                    load_set(sB)
                    compute(sA)
                    load_set(sA)
                    compute(sB)

            if unroll:
                for _ in range(reps // 4):
                    body()
            else:
                engines = tuple(mybir.ALL_ENGINES)
                with tc.For_i(0, reps // 4, 1, hint_engines=engines):
                    body()
    nc.compile()
    return nc


# Gate-dim permutation: position j*4 + t  <-  original gate tile t*4 + j
# (tile index into the stacked-gates dim of 16 x 128 rows).
def _gate_perm():
    perm = np.empty(_G, np.int64)
    pos = 0
    for j in range(_NJ):
        for t in range(_NT):
            src = (t * _NJ + j) * 128
            perm[pos:pos + 128] = np.arange(src, src + 128)
            pos += 128
    return perm


def _bf16():
    import ml_dtypes
    return ml_dtypes.bfloat16


def _host_shards(x, h, c, Wi, bi, Wh, bh):
    bf16 = _bf16()
    perm = _gate_perm()
    W = np.concatenate([np.asarray(Wi, np.float32),
                        np.asarray(Wh, np.float32)], axis=1)    # [G, K]
    wTv = np.ascontiguousarray(W[perm].T.astype(bf16))          # [K, G]
    b = (np.asarray(bi, np.float32) + np.asarray(bh, np.float32))[perm]
    b2 = np.ascontiguousarray(b.reshape(_G // 128, 128).T)      # [128, G/128]
    xh = np.concatenate([np.asarray(x, np.float32),
                         np.asarray(h, np.float32)], axis=1)    # [B, K]
    in_maps = []
    for s in range(_NC):
        sl = slice(s * _BL, (s + 1) * _BL)
        in_maps.append({
            "xhT": np.ascontiguousarray(xh[sl].T.astype(bf16)),
            "cT": np.ascontiguousarray(
                np.asarray(c, np.float32)[sl].T.astype(bf16)),
            "wT": wTv,
            "b2": b2,
        })
    return in_maps


def kernel(x, h, c, Wi, bi, Wh, bh):
    from concourse.bass_utils import run_bass_kernel_spmd

    nc = _cache.get("nc")
    if nc is None:
        nc = _build()
        _cache["nc"] = nc

    in_maps = _host_shards(x, h, c, Wi, bi, Wh, bh)
    res = run_bass_kernel_spmd(nc, in_maps, list(range(_NC)))

    h_out = np.empty((_B, _H), np.float32)
    c_out = np.empty((_B, _H), np.float32)
    for s in range(_NC):
        sl = slice(s * _BL, (s + 1) * _BL)
        h_out[sl] = res.results[s]["hoT"].astype(np.float32).T
        c_out[sl] = res.results[s]["coT"].astype(np.float32).T
    return h_out, c_out
